# revision 1
# baseline (speedup 1.0000x reference)
"""DeepSeek-style MHA (GQA + neox RoPE + causal) on 8 TRN2 NeuronCores.

Sharding: data-parallel over batch (2) x tensor-parallel over kv-heads (4).
Core c handles batch b = c//4, kv head g = c%4, q-heads [4g..4g+4). Each core
computes its 4 heads' attention and a partial o_proj (rows 512g..512g+512 of
w_o); the host sums the 4 partials per batch.

All heavy matmuls run as fp8e4 DoubleRow (2x128-deep contraction per
instruction at 0.5 cycles/row). Full precision is kept with two-plane
(hi/lo) fp8 splits computed on the host:
  x ~= xh + xl,  w ~= (wh + wl)/SCALE   (weights pre-scaled into e4m3 range)
  w.x ~= wh.xh + wl.xh + wh.xl          (three DoubleRow passes)
X arrives pre-transposed from the host ([hid, tok]); no PE transposes at all.
Scores run in bf16 (q/k from RoPE on the DVE). Softmax: exp on the Act engine
(bias -2 for fp8/bf16 range safety), denominator via an appended ones-column
(=64, cancelling the w_qkv quant scale) on the PV stationary so PV+denominator
share PSUM; normalization uses a gpsimd partition_broadcast of 1/denom.
o_proj: attn split hi/lo on the DVE, three DoubleRow passes against host-split
w_o planes; y lands in PSUM, is copied to bf16 and DMA'd out raw (x32 weight
scale removed on the host while summing partials).
"""

import sys

if '/opt/trn_rl_repo' not in sys.path:
    sys.path.insert(0, '/opt/trn_rl_repo')

import numpy as np
from contextlib import ExitStack

B, S, HID = 2, 2048, 2048
NUM_HEADS, NUM_KV_HEADS, D = 16, 4, 128
Q_SIZE = NUM_HEADS * D
KV_SIZE = NUM_KV_HEADS * D
ROPE_THETA = 10000.0
TP = 4                      # kv-head parallel degree
HPC = NUM_HEADS // TP       # q heads per core = 4
NKT = HID // D              # 16 hid tiles
QK_M = HPC + 1              # q0..q3 + k m-tiles
NCH = 4                     # token chunks of 512 for qkv
NQC = 4                     # query chunks of 512 for attention
SCALE = float(1.0 / np.sqrt(D))
WQ_SCALE = 64.0             # e4m3 range scaling for w_qkv (std 1/sqrt(2048))
WO_SCALE = 32.0             # e4m3 range scaling for w_o  (std 1/sqrt(512))

_prog = None


def _build_program():
    import os
    stages = int(os.environ.get("BASS_STAGES", "3"))
    from concourse import bacc, mybir, tile

    F32 = mybir.dt.float32
    BF16 = mybir.dt.bfloat16
    F8 = mybir.dt.float8e4
    AF = mybir.ActivationFunctionType
    ALU = mybir.AluOpType
    DR = mybir.MatmulPerfMode.DoubleRow

    nc = bacc.Bacc("TRN2", target_bir_lowering=False, debug=False)
    xh_d = nc.dram_tensor("xh", [D, NKT, S], F8, kind="ExternalInput").ap()
    xl_d = nc.dram_tensor("xl", [D, NKT, S], F8, kind="ExternalInput").ap()
    wqh_d = nc.dram_tensor("wqh", [D, NKT, QK_M * D], F8, kind="ExternalInput").ap()
    wql_d = nc.dram_tensor("wql", [D, NKT, QK_M * D], F8, kind="ExternalInput").ap()
    wvh_d = nc.dram_tensor("wvh", [D, NKT, D], F8, kind="ExternalInput").ap()
    wvl_d = nc.dram_tensor("wvl", [D, NKT, D], F8, kind="ExternalInput").ap()
    woh_d = nc.dram_tensor("woh", [D, HPC, HID], F8, kind="ExternalInput").ap()
    wol_d = nc.dram_tensor("wol", [D, HPC, HID], F8, kind="ExternalInput").ap()
    cs_d = nc.dram_tensor("cs2", [D, S], BF16, kind="ExternalInput").ap()
    sn_d = nc.dram_tensor("sn2", [D, S], BF16, kind="ExternalInput").ap()
    mask_d = nc.dram_tensor("masks", [D, 4 * 512], BF16, kind="ExternalInput").ap()
    out_d = nc.dram_tensor("out", [S, HID], BF16, kind="ExternalOutput").ap()

    with tile.TileContext(nc) as tc, ExitStack() as octx:
        pers = octx.enter_context(tc.tile_pool(name="pers", bufs=1))

        xh_s = pers.tile([D, NKT, S], F8, tag="xh", name="xh")
        xl_s = pers.tile([D, NKT, S], F8, tag="xl", name="xl")
        wqh_s = pers.tile([D, NKT, QK_M * D], F8, tag="wqh", name="wqh")
        wql_s = pers.tile([D, NKT, QK_M * D], F8, tag="wql", name="wql")
        wvh_s = pers.tile([D, NKT, D], F8, tag="wvh", name="wvh")
        wvl_s = pers.tile([D, NKT, D], F8, tag="wvl", name="wvl")
        woh_s = pers.tile([D, HPC, HID], F8, tag="woh", name="woh")
        wol_s = pers.tile([D, HPC, HID], F8, tag="wol", name="wol")
        cs_s = pers.tile([D, S], BF16, tag="cs", name="cs")
        sn_s = pers.tile([D, S], BF16, tag="sn", name="sn")
        mask_s = pers.tile([D, 4 * 512], BF16, tag="mask", name="mask")
        kT = pers.tile([D, S], BF16, tag="kT", name="kT")
        qT = [pers.tile([D, S], BF16, tag=f"qT{h}", name=f"qT{h}") for h in range(HPC)]
        # v natural layout + ones columns (=WQ_SCALE so the v quant scale
        # cancels against the denominator): per kt [64 d | one | 64 d | one]
        v_nat = pers.tile([D, NKT, 130], BF16, tag="vnat", name="vnat")
        a_hi = pers.tile([D, HPC, S], F8, tag="ahi", name="ahi")
        a_lo = pers.tile([D, HPC, S], F8, tag="alo", name="alo")
        ebias = pers.tile([D, 1], F32, tag="ebias", name="ebias")

        nc.vector.memset(ebias[:], -2.0)
        nc.vector.memset(v_nat[:, :, 64:65], WQ_SCALE)
        nc.vector.memset(v_nat[:, :, 129:130], WQ_SCALE)

        # input DMAs: hi planes (first compute pass) in kt-pair granularity
        # so matmuls can start as soon as pair 0 lands.
        for j in range(NKT // 2):
            js = slice(2 * j, 2 * j + 2)
            nc.sync.dma_start(xh_s[:, js, :], xh_d[:, js, :])
            nc.sync.dma_start(wqh_s[:, js, :], wqh_d[:, js, :])
        nc.sync.dma_start(wvh_s[:], wvh_d)
        nc.sync.dma_start(cs_s[:], cs_d)
        nc.sync.dma_start(sn_s[:], sn_d)
        for j in range(NKT // 2):
            js = slice(2 * j, 2 * j + 2)
            nc.sync.dma_start(xl_s[:, js, :], xl_d[:, js, :])
            nc.sync.dma_start(wql_s[:, js, :], wql_d[:, js, :])
        nc.sync.dma_start(wvl_s[:], wvl_d)
        nc.sync.dma_start(mask_s[:], mask_d)
        nc.sync.dma_start(woh_s[:], woh_d)
        nc.sync.dma_start(wol_s[:], wol_d)

        # ---------------- Stage A: QKV projection + RoPE + V ----------------
        with ExitStack() as sA:
            pA = sA.enter_context(tc.tile_pool(name="pA", bufs=2, space="PSUM"))
            rp = sA.enter_context(tc.tile_pool(name="rp", bufs=2))

            qk_passes = [(wqh_s, xh_s), (wql_s, xh_s), (wqh_s, xl_s)]
            v_passes = [(xh_s, wvh_s), (xh_s, wvl_s), (xl_s, wvh_s)]

            for ch in range(NCH):
                chsl = slice(ch * 512, (ch + 1) * 512)
                for m in range(QK_M):
                    qp = pA.tile([D, 512], F32, tag="qp", name="qp")
                    n = 0
                    for wt, xt in qk_passes:
                        for j in range(NKT // 2):
                            js = slice(2 * j, 2 * j + 2)
                            nc.tensor.matmul(
                                qp[:], wt[:, js, m * D:(m + 1) * D], xt[:, js, chsl],
                                start=(n == 0), stop=(n == 23), perf_mode=DR)
                            n += 1
                    # rope: out = x*cs2 + swap(x*sn2), sn2 = [sin; -sin] so
                    # the half-swap lands the signed cross terms correctly.
                    e_t = rp.tile([D, 512], BF16, tag="e", name="e")
                    nc.vector.tensor_copy(e_t[:], qp[:])
                    ra = rp.tile([D, 512], BF16, tag="ra", name="ra")
                    rb = rp.tile([D, 512], BF16, tag="rb", name="rb")
                    nc.vector.tensor_tensor(ra[:], e_t[:], cs_s[:, chsl], ALU.mult)
                    nc.vector.tensor_tensor(rb[:], e_t[:], sn_s[:, chsl], ALU.mult)
                    rbs = rp.tile([D, 512], BF16, tag="rbs", name="rbs")
                    nc.sync.dma_start(rbs[0:64, :], rb[64:128, :])
                    nc.sync.dma_start(rbs[64:128, :], rb[0:64, :])
                    dest = qT[m] if m < HPC else kT
                    nc.vector.tensor_tensor(dest[0:64, chsl], ra[0:64, :],
                                            rbs[0:64, :], ALU.add)
                    nc.vector.tensor_tensor(dest[64:128, chsl], ra[64:128, :],
                                            rbs[64:128, :], ALU.add)
                for t in range(4 * ch, 4 * ch + 4):
                    vp = pA.tile([D, D], F32, tag="vp", name="vp")
                    n = 0
                    for xt, wt in v_passes:
                        for j in range(NKT // 2):
                            js = slice(2 * j, 2 * j + 2)
                            nc.tensor.matmul(
                                vp[:], xt[:, js, t * D:(t + 1) * D], wt[:, js, :],
                                start=(n == 0), stop=(n == 23), perf_mode=DR)
                            n += 1
                    nc.scalar.copy(v_nat[:, t, 0:64], vp[:, 0:64])
                    nc.scalar.copy(v_nat[:, t, 65:129], vp[:, 64:128])

        if stages < 2:
            for ncx in range(4):
                csl = slice(ncx * 512, (ncx + 1) * 512)
                for sdx, src_t in enumerate([kT, qT[0], qT[3]]):
                    nc.sync.dma_start(out_d[sdx * D:(sdx + 1) * D, csl], src_t[:, csl])
            for t in range(NKT):
                nc.sync.dma_start(out_d[3 * D:4 * D, t * 128:(t + 1) * 128],
                                  v_nat[:, t, 0:128])

        # ---------------- Stage B: attention + o_proj (qc-pipelined) --------
        if stages < 2:
            nc.compile()
            return nc
        pSC = octx.enter_context(tc.tile_pool(name="pSC", bufs=1, space="PSUM"))
        pPV = octx.enter_context(tc.tile_pool(name="pPV", bufs=2, space="PSUM"))
        pYP = octx.enter_context(tc.tile_pool(name="pYP", bufs=2, space="PSUM"))
        ptp = octx.enter_context(tc.tile_pool(name="ptp", bufs=2))
        nrm = octx.enter_context(tc.tile_pool(name="nrm", bufs=2))
        ybp = octx.enter_context(tc.tile_pool(name="ybp", bufs=4))

        def emit_pv(pv0, pv1, pt, kts, nblk):
            for i, kt in enumerate(kts):
                psl = slice(i * 512, (i + 1) * 512)
                nc.tensor.matmul(pv0[0:65, :], v_nat[:, kt, 0:65], pt[:, psl],
                                 start=(kt == 0), stop=(kt == nblk - 1))
                nc.tensor.matmul(pv1[0:65, :], v_nat[:, kt, 65:130], pt[:, psl],
                                 start=(kt == 0), stop=(kt == nblk - 1))

        def emit_oproj(qc):
            for t in range(4 * qc, 4 * qc + 4):
                tsl = slice(t * D, (t + 1) * D)
                for cc in range(4):
                    ccsl = slice(cc * 512, (cc + 1) * 512)
                    yp = pYP.tile([D, 512], F32, tag="yp", name="yp")
                    n = 0
                    for A, W in ((a_hi, woh_s), (a_hi, wol_s), (a_lo, woh_s)):
                        for j in range(2):
                            js = slice(2 * j, 2 * j + 2)
                            nc.tensor.matmul(
                                yp[:], A[:, js, tsl], W[:, js, ccsl],
                                start=(n == 0), stop=(n == 5), perf_mode=DR)
                            n += 1
                    yt = ybp.tile([D, 512], BF16, tag="yt", name="yt")
                    if (t + cc) % 2 == 0:
                        nc.vector.tensor_copy(yt[:], yp[:])
                    else:
                        nc.scalar.copy(yt[:], yp[:])
                    nc.sync.dma_start(out_d[tsl, ccsl], yt[:])

        for qc in range(NQC):
            nblk = 4 * qc + 4
            qsl = slice(qc * 512, (qc + 1) * 512)
            for h in range(HPC):
                pv0 = pPV.tile([D, 512], F32, tag="pv", name="pv0")
                pv1 = pPV.tile([D, 512], F32, tag="pv", name="pv1")
                prev = None
                for g0 in range(0, nblk, 4):
                    kts = list(range(g0, g0 + 4))
                    sc = pSC.tile([D, 2048], F32, tag="sc", name="sc")
                    for i, kt in enumerate(kts):
                        nc.tensor.matmul(
                            sc[:, i * 512:(i + 1) * 512],
                            kT[:, kt * D:(kt + 1) * D], qT[h][:, qsl],
                            start=True, stop=True)
                    pt = ptp.tile([D, 2048], BF16, tag="pt", name="pt")
                    nc.scalar.activation(pt[:], sc[:], AF.Exp, bias=ebias[:],
                                         scale=SCALE / (WQ_SCALE * WQ_SCALE))
                    for i, kt in enumerate(kts):
                        r = kt - 4 * qc
                        if r >= 0:
                            psl = slice(i * 512, (i + 1) * 512)
                            nc.vector.tensor_tensor(
                                pt[:, psl], pt[:, psl],
                                mask_s[:, r * 512:(r + 1) * 512], ALU.mult)
                    if prev is not None:
                        emit_pv(pv0, pv1, *prev, nblk)
                    prev = (pt, kts)
                emit_pv(pv0, pv1, *prev, nblk)
                # normalize + hi/lo split for o_proj
                rc = nrm.tile([1, 512], F32, tag="rc", name="rc")
                nc.vector.reciprocal(rc[:], pv0[64:65, :])
                rcb = nrm.tile([D, 512], F32, tag="rcb", name="rcb")
                nc.gpsimd.partition_broadcast(rcb[:], rc[:])
                t_bf = nrm.tile([D, 512], BF16, tag="tbf", name="tbf")
                nc.vector.tensor_tensor(t_bf[0:64, :], pv0[0:64, :],
                                        rcb[0:64, :], ALU.mult)
                nc.vector.tensor_tensor(t_bf[64:128, :], pv1[0:64, :],
                                        rcb[64:128, :], ALU.mult)
                nc.gpsimd.tensor_copy(a_hi[:, h, qsl], t_bf[:])
                nc.vector.tensor_tensor(a_lo[:, h, qsl], t_bf[:],
                                        a_hi[:, h, qsl], ALU.subtract)
            if qc > 0:
                emit_oproj(qc - 1)
        emit_oproj(NQC - 1)

    nc.compile()
    return nc


def _get_program():
    global _prog
    if _prog is None:
        _prog = _build_program()
    return _prog


def _f8(x):
    import ml_dtypes
    return np.ascontiguousarray(x).astype(ml_dtypes.float8_e4m3)


def _hl(x, scale):
    """Two-plane e4m3 split of x*scale (hi + lo ~= x*scale to ~0.1%)."""
    import ml_dtypes
    xs = (x * scale).astype(np.float32)
    hi = xs.astype(ml_dtypes.float8_e4m3)
    lo = (xs - hi.astype(np.float32)).astype(ml_dtypes.float8_e4m3)
    return np.ascontiguousarray(hi), np.ascontiguousarray(lo)


def _host_tables(positions_b):
    import ml_dtypes
    inv_freq = (1.0 / (ROPE_THETA ** (np.arange(0, D, 2, dtype=np.float32) / D)))
    ang = positions_b.astype(np.float32)[:, None] * inv_freq[None, :].astype(np.float32)
    cosT = np.cos(ang).T.astype(np.float32)
    sinT = np.sin(ang).T.astype(np.float32)
    cs2 = np.concatenate([cosT, cosT], axis=0)
    sn2 = np.concatenate([sinT, -sinT], axis=0)
    return (np.ascontiguousarray(cs2.astype(ml_dtypes.bfloat16)),
            np.ascontiguousarray(sn2.astype(ml_dtypes.bfloat16)))


def _host_masks():
    import ml_dtypes
    k = np.arange(D)[:, None]
    j = np.arange(512)[None, :]
    pats = [((m * D + k) <= j).astype(np.float32) for m in range(4)]
    masks = np.concatenate(pats, axis=1)
    return np.ascontiguousarray(masks.astype(ml_dtypes.bfloat16))


def kernel(positions, hidden_states, w_qkv, w_o):
    from concourse.bass_utils import run_bass_kernel_spmd

    nc = _get_program()

    positions = np.asarray(positions)
    hidden_states = np.asarray(hidden_states, dtype=np.float32)
    w_qkv = np.asarray(w_qkv, dtype=np.float32)
    w_o = np.asarray(w_o, dtype=np.float32)

    masks = _host_masks()
    tables = [_host_tables(positions[b]) for b in range(B)]
    # X^T in [128, kt, tok] layout, split into e4m3 hi/lo planes (per batch)
    xhl = []
    for b in range(B):
        xt = hidden_states[b].T.reshape(NKT, D, S).transpose(1, 0, 2)
        xhl.append(_hl(xt, 1.0))

    in_maps = []
    for c in range(2 * TP):
        b, g = c // TP, c % TP
        wq_cols = np.concatenate([
            w_qkv[:, g * HPC * D:(g + 1) * HPC * D],           # 4 q heads
            w_qkv[:, Q_SIZE + g * D: Q_SIZE + (g + 1) * D],    # k head g
        ], axis=1)                                             # [2048, 640]
        wv_col = w_qkv[:, Q_SIZE + KV_SIZE + g * D: Q_SIZE + KV_SIZE + (g + 1) * D]
        wqh, wql = _hl(wq_cols.reshape(NKT, D, QK_M * D).transpose(1, 0, 2), WQ_SCALE)
        wvh, wvl = _hl(wv_col.reshape(NKT, D, D).transpose(1, 0, 2), WQ_SCALE)
        wo_sl = w_o[g * HPC * D:(g + 1) * HPC * D, :]          # [512, 2048]
        woh, wol = _hl(wo_sl.reshape(HPC, D, HID).transpose(1, 0, 2), WO_SCALE)
        cs2, sn2 = tables[b]
        xh, xl = xhl[b]
        in_maps.append({
            "xh": xh, "xl": xl,
            "wqh": wqh, "wql": wql, "wvh": wvh, "wvl": wvl,
            "woh": woh, "wol": wol,
            "cs2": cs2, "sn2": sn2, "masks": masks,
        })

    res = run_bass_kernel_spmd(nc, in_maps, core_ids=list(range(2 * TP)))

    out = np.zeros((B, S, HID), dtype=np.float32)
    for c in range(2 * TP):
        b = c // TP
        out[b] += res.results[c]["out"].astype(np.float32)
    out *= 1.0 / WO_SCALE
    return out

